# revision 12
# baseline (speedup 1.0000x reference)
"""Trainium2 Bass kernel for nn_BaselineMNISTClassifier (vq_codebook).

reference:
    x = samples - 0.5                        # [B, F]
    hv = einsum('bf,df->bd', x, bhv)         # [B, D]
    e = (hv > 0)                             # binary
    ham[b, c] = sum_d |e - centroids[c, d]|  # [B, C]
    return -ham

Identity used on device: with s = sign(hv) in {-1, +1} and
cmod = 1 - 2c in {-1, +1}:  |e - c| = s * cmod / 2 + 1/2, so
    ham[b, c] = (sum_d s[b, d] * cmod[c, d]) / 2 + D/2
which turns the broadcast Hamming into a second (tiny) matmul.

Sharding: D axis (10000) split across 8 cores, 1250 (padded to 1280) per
core. Each core computes full-batch partial hamming [C, B]; partials sum
on the host (padded dims contribute exactly 0: centroid pad value 0.5
makes cmod = 0 there, and sign(0) = 0 besides).

Encode matmul runs in float32r (~tf32 precision, full PE rate at N=512).
Inputs ship as bf16 hi/lo pairs (half the HBM traffic of fp32; hi+lo
carries ~16 mantissa bits, more than fp32r keeps) and are reconstructed
to f32r on the DVE: x = (hi - 0.5) + lo in one op. Both operands are
host-transposed so the contraction dim F sits on SBUF partitions.
Second matmul runs in bf16 (s, cmod exact in bf16).

Perf structure (per core):
  - warmup matmuls release the PE HAM clock gate while inputs stream
  - few, large DMAs (~0.5 MB) since each trigger costs ~0.6 us on its
    issuing engine; triggers spread across SP (x_hi), Scalar (x_lo) and
    GpSimd (w, centroids, outputs)
  - fi-outer/bb-inner matmul order: 4 consecutive matmuls share weights
  - all four hamming accumulators live in ONE PSUM bank at partition
    offsets 0/32/64/96; the hamming matmuls are col-tiled
    (tile_position) so they overlap on the PE array; 7 PSUM banks feed
    the encode accumulation (deep multi-buffering)
  - binarize (Sign) on the Scalar engine, reconstruction on DVE —
    no single engine is near the PE's utilization
  - hamming matmuls emitted one d-tile late so PE never waits on the
    binarize; epilogue on Scalar, output DMA per b-block immediately
"""

import sys

sys.path.insert(0, "/opt/trn_rl_repo")

import numpy as np

import concourse.bacc as bacc
import concourse.bass as bass
import concourse.mybir as mybir
import concourse.tile as tile
from concourse.bass_utils import run_bass_kernel_spmd

B = 4096
F = 784
D = 10000
C = 10
NCORES = 8
DREAL = D // NCORES          # 1250 real dims per core
DP = 1280                    # padded to 10 d-tiles of 128
ND = DP // 128               # 10
NB = B // 512                # 8 b-blocks of 512
NBG = 2                      # b-groups of 4 blocks
FT = [(i * 128, min(128, F - i * 128)) for i in range((F + 127) // 128)]
NF = len(FT)                 # 7 (6x128 + 16)
NWARM = 28                   # PE warmup matmuls

F32 = mybir.dt.float32
F32R = mybir.dt.float32r
BF16 = mybir.dt.bfloat16
OP = mybir.AluOpType
AF = mybir.ActivationFunctionType

_NC_CACHE = {}


def _build_nc():
    if "nc" in _NC_CACHE:
        return _NC_CACHE["nc"]
    nc = bacc.Bacc("TRN2", debug=False, target_bir_lowering=False)
    x_hi = nc.dram_tensor("x_hi", [F, B], BF16, kind="ExternalInput")
    x_lo = nc.dram_tensor("x_lo", [F, B], BF16, kind="ExternalInput")
    wT = nc.dram_tensor("wT", [F, DP], F32R, kind="ExternalInput")
    cT = nc.dram_tensor("cT", [DP, C], F32, kind="ExternalInput")
    out = nc.dram_tensor("out", [C, B], F32, kind="ExternalOutput")

    with tile.TileContext(nc) as tc:
        with (
            tc.tile_pool(name="dum", bufs=2) as dumpool,
            tc.tile_pool(name="xhl", bufs=3) as xhlpool,
            tc.tile_pool(name="xp", bufs=NB // 4 * NF) as xpool,
            tc.tile_pool(name="wp", bufs=NF) as wpool,
            tc.tile_pool(name="cp", bufs=1) as cpool,
            tc.tile_pool(name="cmp", bufs=1) as cmpool,
            tc.tile_pool(name="ep", bufs=6) as epool,
            tc.tile_pool(name="op", bufs=4) as opool,
            tc.tile_pool(name="pse", bufs=7, space="PSUM") as psepool,
            tc.tile_pool(name="ps2", bufs=1, space="PSUM") as ps2pool,
        ):
            # --- PE warmup: release the HAM clock gate while inputs load.
            wdum = dumpool.tile([128, 128], BF16)
            nc.gpsimd.memset(wdum[:], 1.0)
            xdum = dumpool.tile([128, 512], BF16)
            nc.gpsimd.memset(xdum[:], 1.0)
            psdum = psepool.tile([128, 512], F32, name="psdum", tag="pse")
            for i in range(NWARM):
                nc.tensor.matmul(psdum[:], wdum[:], xdum[:],
                                 start=(i == 0), stop=(i == NWARM - 1))

            # --- centroid prep: one DMA for all 10 d-tiles, then
            # cmod = 1 - 2c (bf16). Pad rows are 0.5 -> cmod = 0.
            ct = cpool.tile([128, ND * C], F32)
            nc.gpsimd.dma_start(
                ct[:].rearrange("p (a c) -> p a c", c=C),
                cT.ap().rearrange("(a p) c -> p a c", p=128))
            cmod = cmpool.tile([128, ND * C], BF16)
            nc.scalar.activation(cmod[:], ct[:], AF.Copy, bias=1.0,
                                 scale=-2.0)
            cmods = [cmod[:, di * C:(di + 1) * C] for di in range(ND)]

            # --- input loads; reconstructed f32r tiles are
            # single-assignment (loaded once, no slot reuse). x tiles span
            # a whole b-group (4 blocks), w tiles the full d range.
            xts = {}
            wts = {}

            def load_x(bg, fi):
                f0, fl = FT[fi]
                xh = xhlpool.tile([fl, 2048], BF16, name=f"xh_{bg}_{fi}",
                                  tag="xh")
                nc.sync.dma_start(
                    xh[:], x_hi[f0:f0 + fl, bg * 2048:(bg + 1) * 2048])
                xl = xhlpool.tile([fl, 2048], BF16, name=f"xl_{bg}_{fi}",
                                  tag="xl")
                nc.scalar.dma_start(
                    xl[:], x_lo[f0:f0 + fl, bg * 2048:(bg + 1) * 2048])
                xt = xpool.tile([fl, 2048], F32R, name=f"xt_{bg}_{fi}",
                                tag="xt")
                # x = hi + lo (centering folded into the host encoding);
                # alternate engines so reconstruction parallelizes
                eng = nc.vector if fi % 2 == 0 else nc.gpsimd
                eng.tensor_add(xt[:], xh[:], xl[:])
                xts[bg, fi] = xt

            def load_w(fi):
                f0, fl = FT[fi]
                wt = wpool.tile([fl, DP], F32R, name=f"wt_{fi}", tag="wt")
                nc.gpsimd.dma_start(wt[:], wT[f0:f0 + fl, :])
                wts[fi] = wt

            for fi in range(NF):
                load_x(0, fi)
                load_w(fi)
            for fi in range(NF):
                load_x(1, fi)

            def xop(bb, fi):
                return xts[bb // 4, fi][:, (bb % 4) * 512:(bb % 4 + 1) * 512]

            def wop(di, fi):
                return wts[fi][:, di * 128:(di + 1) * 128]

            # --- main compute: two b-groups of 4 blocks.
            for bg in range(NBG):
                bbs = list(range(bg * 4, bg * 4 + 4))
                ps2 = ps2pool.tile([128, 512], F32, name=f"ps2_{bg}",
                                   tag="ps2")
                p2 = {bb: ps2[32 * (bb % 4):32 * (bb % 4) + C, :]
                      for bb in bbs}
                pending = []
                for di in range(ND):
                    pses = {}
                    for bb in bbs:
                        pses[bb] = psepool.tile([128, 512], F32,
                                                name=f"pse_{di % 2}_{bb}",
                                                tag="pse")
                    for fi in range(NF):
                        for bb in bbs:
                            nc.tensor.matmul(pses[bb][:], wop(di, fi),
                                             xop(bb, fi),
                                             start=(fi == 0),
                                             stop=(fi == NF - 1))
                    ets = {}
                    for bb in bbs:
                        # s = sign(hv) in {-1, +1} (exact zeros only on
                        # padded dims, where cmod = 0 anyway)
                        et = epool.tile([128, 512], BF16,
                                        name=f"et_{di % 2}_{bb}", tag="et")
                        nc.scalar.activation(et[:], pses[bb][:], AF.Sign)
                        ets[bb] = et
                    # hamming matmuls one d-tile late (PE never stalls on
                    # the binarize), col-tiled to overlap on the PE array
                    for pdi, pbb, pet in pending:
                        nc.tensor.matmul(p2[pbb], cmods[pdi], pet[:],
                                         start=(pdi == 0),
                                         stop=(pdi == ND - 1),
                                         tile_position=(0, 32 * (pbb % 4)))
                    pending = [(di, bb, ets[bb]) for bb in bbs]
                for pdi, pbb, pet in pending:
                    nc.tensor.matmul(p2[pbb], cmods[pdi], pet[:],
                                     start=(pdi == 0), stop=(pdi == ND - 1),
                                     tile_position=(0, 32 * (pbb % 4)))
                    # out = -(psum2/2 + DREAL/2), on the Scalar engine
                    bb = pbb
                    ot = opool.tile([C, 512], F32, name=f"ot_{bb % 4}",
                                    tag="ot")
                    nc.scalar.activation(ot[:], p2[pbb], AF.Copy,
                                         bias=-float(DREAL) / 2.0,
                                         scale=-0.5)
                    nc.gpsimd.dma_start(
                        out[:, bb * 512:(bb + 1) * 512], ot[:])
    nc.compile()
    _NC_CACHE["nc"] = nc
    return nc


def _prep_in_maps(samples, bhv_matrix, centroids):
    import ml_dtypes
    bf16 = ml_dtypes.bfloat16
    samples = np.ascontiguousarray(samples, dtype=np.float32)
    bhv_matrix = np.ascontiguousarray(bhv_matrix, dtype=np.float32)
    centroids = np.ascontiguousarray(centroids, dtype=np.float32)

    xc = np.ascontiguousarray(samples.T) - np.float32(0.5)  # [F, B] f32
    x_hi = xc.astype(bf16)
    x_lo = (xc - x_hi.astype(np.float32)).astype(bf16)

    in_maps = []
    for k in range(NCORES):
        lo_, hi_ = k * DREAL, (k + 1) * DREAL
        wTk = np.zeros((F, DP), dtype=np.float32)
        wTk[:, :DREAL] = bhv_matrix[lo_:hi_, :].T
        cTk = np.full((DP, C), 0.5, dtype=np.float32)
        cTk[:DREAL, :] = centroids[:, lo_:hi_].T
        in_maps.append({"x_hi": x_hi, "x_lo": x_lo,
                        "wT": wTk, "cT": cTk})
    return in_maps


def _run(samples, bhv_matrix, centroids, **spmd_kwargs):
    nc = _build_nc()
    in_maps = _prep_in_maps(samples, bhv_matrix, centroids)
    res = run_bass_kernel_spmd(nc, in_maps, core_ids=list(range(NCORES)),
                               **spmd_kwargs)
    acc = np.zeros((C, B), dtype=np.float32)
    for r in res.results:
        acc += r["out"]
    return np.ascontiguousarray(acc.T), res


def kernel(samples, bhv_matrix, centroids):
    out, _ = _run(samples, bhv_matrix, centroids)
    return out


# revision 13
# speedup vs baseline: 1.0775x; 1.0775x over previous
"""Trainium2 Bass kernel for nn_BaselineMNISTClassifier (vq_codebook).

reference:
    x = samples - 0.5                        # [B, F]
    hv = einsum('bf,df->bd', x, bhv)         # [B, D]
    e = (hv > 0)                             # binary
    ham[b, c] = sum_d |e - centroids[c, d]|  # [B, C]
    return -ham

Identity used on device: with s = sign(hv) in {-1, +1} and
cmod = 1 - 2c in {-1, +1}:  |e - c| = s * cmod / 2 + 1/2, so
    ham[b, c] = (sum_d s[b, d] * cmod[c, d]) / 2 + D/2
which turns the broadcast Hamming into a second (tiny) matmul.

Sharding: D axis (10000) split across 8 cores, 1250 (padded to 1280) per
core. Each core computes full-batch partial hamming [C, B]; partials sum
on the host (padded dims contribute exactly 0: centroid pad value 0.5
makes cmod = 0 there, and sign(0) = 0 besides).

Encode matmul runs in float32r (~tf32 precision, full PE rate at N=512).
Both operands are host-transposed so the contraction dim F sits on SBUF
partitions. Second matmul runs in bf16 (s, cmod exact in bf16).

Perf structure (per core):
  - warmup matmuls release the PE HAM clock gate while inputs stream;
    extra dummy fills inside the first (DMA-bound) d-tile group keep
    the clock warm through the ramp
  - x DMA triggers alternate between the SP and Scalar queues (each
    trigger costs ~0.6 us of issue time); w/centroid/output triggers go
    to GpSimd
  - four b-groups of 2 blocks: the first group's encode only needs
    3.5 MB of x before it can run at full rate
  - both hamming accumulators of a group live in ONE PSUM bank at
    partition offsets 0/32, col-tiled (tile_position) so they overlap
    on the PE array; 6 PSUM banks feed the encode accumulation
  - binarize (Sign) on the Scalar engine, centering on DVE
  - hamming matmuls emitted one d-tile late so PE never waits on the
    binarize; epilogue on Scalar, output DMA per b-block immediately
"""

import sys

sys.path.insert(0, "/opt/trn_rl_repo")

import numpy as np

import concourse.bacc as bacc
import concourse.bass as bass
import concourse.mybir as mybir
import concourse.tile as tile
from concourse.bass_utils import run_bass_kernel_spmd

B = 4096
F = 784
D = 10000
C = 10
NCORES = 8
DREAL = D // NCORES          # 1250 real dims per core
DP = 1280                    # padded to 10 d-tiles of 128
ND = DP // 128               # 10
NB = B // 512                # 8 b-blocks of 512
FT = [(i * 128, min(128, F - i * 128)) for i in range((F + 127) // 128)]
NF = len(FT)                 # 7 (6x128 + 16)
NWARM = 28                   # PE warmup matmuls
NFILL = 5                    # dummy matmuls per fi-step of the first group

F32 = mybir.dt.float32
F32R = mybir.dt.float32r
BF16 = mybir.dt.bfloat16
OP = mybir.AluOpType
AF = mybir.ActivationFunctionType

_NC_CACHE = {}


def _build_nc():
    if "nc" in _NC_CACHE:
        return _NC_CACHE["nc"]
    nc = bacc.Bacc("TRN2", debug=False, target_bir_lowering=False)
    xT = nc.dram_tensor("xT", [F, B], F32R, kind="ExternalInput")
    wT = nc.dram_tensor("wT", [F, DP], F32R, kind="ExternalInput")
    cT = nc.dram_tensor("cT", [DP, C], F32, kind="ExternalInput")
    out = nc.dram_tensor("out", [C, B], F32, kind="ExternalOutput")

    with tile.TileContext(nc) as tc:
        with (
            tc.tile_pool(name="dum", bufs=2) as dumpool,
            tc.tile_pool(name="xp", bufs=NB // 2 * NF) as xpool,
            tc.tile_pool(name="wp", bufs=(ND + 1) // 2 * NF) as wpool,
            tc.tile_pool(name="cp", bufs=1) as cpool,
            tc.tile_pool(name="cmp", bufs=1) as cmpool,
            tc.tile_pool(name="ep", bufs=6) as epool,
            tc.tile_pool(name="op", bufs=4) as opool,
            tc.tile_pool(name="pse", bufs=6, space="PSUM") as psepool,
            tc.tile_pool(name="ps2", bufs=2, space="PSUM") as ps2pool,
        ):
            # --- PE warmup: release the HAM clock gate while inputs load.
            wdum = dumpool.tile([128, 128], BF16)
            nc.gpsimd.memset(wdum[:], 1.0)
            xdum = dumpool.tile([128, 512], BF16)
            nc.gpsimd.memset(xdum[:], 1.0)
            psdum = psepool.tile([128, 512], F32, name="psdum", tag="pse")
            for i in range(NWARM):
                nc.tensor.matmul(psdum[:], wdum[:], xdum[:],
                                 start=(i == 0), stop=(i == NWARM - 1))

            def pe_fill(n, key):
                """Dummy matmuls to keep the PE clock warm where the
                stream is DMA-bound."""
                ps = psepool.tile([128, 512], F32, name=f"fill_{key}",
                                  tag="pse")
                for i in range(n):
                    nc.tensor.matmul(ps[:], wdum[:], xdum[:],
                                     start=(i == 0), stop=(i == n - 1))

            # --- centroid prep: one DMA for all 10 d-tiles, then
            # cmod = 1 - 2c (bf16). Pad rows are 0.5 -> cmod = 0.
            ct = cpool.tile([128, ND * C], F32)
            nc.gpsimd.dma_start(
                ct[:].rearrange("p (a c) -> p a c", c=C),
                cT.ap().rearrange("(a p) c -> p a c", p=128))
            cmod = cmpool.tile([128, ND * C], BF16)
            nc.scalar.activation(cmod[:], ct[:], AF.Copy, bias=1.0,
                                 scale=-2.0)
            cmods = [cmod[:, di * C:(di + 1) * C] for di in range(ND)]

            # --- input loads; tiles single-assignment (loaded once, no
            # slot reuse) so input DMAs never carry data waits. x tiles
            # span two b-blocks, w tiles two d-tiles.
            xts = {}
            wts = {}

            def load_x(bp, fi):   # bp = b-block pair index (0..3)
                f0, fl = FT[fi]
                xt = xpool.tile([fl, 1024], F32R, name=f"xt_{bp}_{fi}",
                                tag="xt")
                eng = nc.sync if fi % 2 == 0 else nc.scalar
                eng.dma_start(
                    xt[:], xT[f0:f0 + fl, bp * 1024:(bp + 1) * 1024])
                # center (x - 0.5) in place on DVE
                nc.vector.tensor_scalar_add(xt[:], xt[:], -0.5)
                xts[bp, fi] = xt

            def load_w(dp, fi):   # dp = d-tile pair index (0..4)
                f0, fl = FT[fi]
                wid = min(256, DP - dp * 256)
                wt = wpool.tile([fl, wid], F32R, name=f"wt_{dp}_{fi}",
                                tag="wt")
                nc.gpsimd.dma_start(
                    wt[:], wT[f0:f0 + fl, dp * 256:dp * 256 + wid])
                wts[dp, fi] = wt

            for i in range(5):
                for fi in range(NF):
                    if i < 4:
                        load_x(i, fi)
                    load_w(i, fi)

            def xop(bb, fi):
                return xts[bb // 2, fi][:, (bb % 2) * 512:(bb % 2 + 1) * 512]

            def wop(di, fi):
                return wts[di // 2, fi][:, (di % 2) * 128:(di % 2 + 1) * 128]

            # --- main compute: four b-groups of 2 blocks.
            for bg in range(4):
                ps2 = ps2pool.tile([128, 512], F32, name=f"ps2_{bg % 2}",
                                   tag="ps2")
                p2 = {0: ps2[0:C, :], 1: ps2[32:32 + C, :]}
                pending = []
                for di in range(ND):
                    pses = {}
                    for j in range(2):
                        pses[j] = psepool.tile([128, 512], F32,
                                               name=f"pse_{di % 2}_{j}",
                                               tag="pse")
                    for fi in range(NF):
                        for j in range(2):
                            nc.tensor.matmul(pses[j][:], wop(di, fi),
                                             xop(2 * bg + j, fi),
                                             start=(fi == 0),
                                             stop=(fi == NF - 1))
                        if bg == 0 and di == 0 and fi < NF - 1:
                            pe_fill(NFILL, f"r{fi}")
                    ets = {}
                    for j in range(2):
                        # s = sign(hv) in {-1, +1} (exact zeros only on
                        # padded dims, where cmod = 0 anyway)
                        et = epool.tile([128, 512], BF16,
                                        name=f"et_{di % 2}_{j}", tag="et")
                        nc.scalar.activation(et[:], pses[j][:], AF.Sign)
                        ets[j] = et
                    # hamming matmuls one d-tile late (PE never stalls on
                    # the binarize), col-tiled to overlap on the PE array
                    for pdi, pj, pet in pending:
                        nc.tensor.matmul(p2[pj], cmods[pdi], pet[:],
                                         start=(pdi == 0),
                                         stop=(pdi == ND - 1),
                                         tile_position=(0, 32 * pj))
                    pending = [(di, j, ets[j]) for j in range(2)]
                for pdi, pj, pet in pending:
                    nc.tensor.matmul(p2[pj], cmods[pdi], pet[:],
                                     start=(pdi == 0), stop=(pdi == ND - 1),
                                     tile_position=(0, 32 * pj))
                    # out = -(psum2/2 + DREAL/2), on the Scalar engine
                    bb = 2 * bg + pj
                    ot = opool.tile([C, 512], F32, name=f"ot_{bb % 4}",
                                    tag="ot")
                    nc.scalar.activation(ot[:], p2[pj], AF.Copy,
                                         bias=-float(DREAL) / 2.0,
                                         scale=-0.5)
                    nc.gpsimd.dma_start(
                        out[:, bb * 512:(bb + 1) * 512], ot[:])
    nc.compile()
    _NC_CACHE["nc"] = nc
    return nc


def _prep_in_maps(samples, bhv_matrix, centroids):
    samples = np.ascontiguousarray(samples, dtype=np.float32)
    bhv_matrix = np.ascontiguousarray(bhv_matrix, dtype=np.float32)
    centroids = np.ascontiguousarray(centroids, dtype=np.float32)
    xT = np.ascontiguousarray(samples.T)  # [F, B]
    in_maps = []
    for k in range(NCORES):
        lo_, hi_ = k * DREAL, (k + 1) * DREAL
        wTk = np.zeros((F, DP), dtype=np.float32)
        wTk[:, :DREAL] = bhv_matrix[lo_:hi_, :].T
        cTk = np.full((DP, C), 0.5, dtype=np.float32)
        cTk[:DREAL, :] = centroids[:, lo_:hi_].T
        in_maps.append({"xT": xT, "wT": wTk, "cT": cTk})
    return in_maps


def _run(samples, bhv_matrix, centroids, **spmd_kwargs):
    nc = _build_nc()
    in_maps = _prep_in_maps(samples, bhv_matrix, centroids)
    res = run_bass_kernel_spmd(nc, in_maps, core_ids=list(range(NCORES)),
                               **spmd_kwargs)
    acc = np.zeros((C, B), dtype=np.float32)
    for r in res.results:
        acc += r["out"]
    return np.ascontiguousarray(acc.T), res


def kernel(samples, bhv_matrix, centroids):
    out, _ = _run(samples, bhv_matrix, centroids)
    return out


# revision 14
# speedup vs baseline: 1.1570x; 1.0738x over previous
"""Trainium2 Bass kernel for nn_BaselineMNISTClassifier (vq_codebook).

reference:
    x = samples - 0.5                        # [B, F]
    hv = einsum('bf,df->bd', x, bhv)         # [B, D]
    e = (hv > 0)                             # binary
    ham[b, c] = sum_d |e - centroids[c, d]|  # [B, C]
    return -ham

Identity used on device: with s = sign(hv) in {-1, +1} and
cmod = 1 - 2c in {-1, +1}:  |e - c| = s * cmod / 2 + 1/2, so
    ham[b, c] = (sum_d s[b, d] * cmod[c, d]) / 2 + D/2
which turns the broadcast Hamming into a second (tiny) matmul.

Sharding: D axis (10000) split across 8 cores, 1250 (padded to 1280) per
core. Each core computes full-batch partial hamming [C, B]; partials sum
on the host (padded dims contribute exactly 0: centroid pad value 0.5
makes cmod = 0 there, and sign(0) = 0 besides).

Encode matmul runs in float32r (~tf32 precision, full PE rate at N=512).
Both operands are host-transposed so the contraction dim F sits on SBUF
partitions. Second matmul runs in bf16 (s, cmod exact in bf16).

Perf structure (per core):
  - warmup matmuls release the PE HAM clock gate while inputs stream;
    extra dummy fills inside the first (DMA-bound) d-tile group keep
    the clock warm through the ramp
  - x DMA triggers alternate between the SP and Scalar queues (each
    trigger costs ~0.6 us of issue time); w/centroid/output triggers go
    to GpSimd
  - four b-groups of 2 blocks: the first group's encode only needs
    3.5 MB of x before it can run at full rate
  - both hamming accumulators of a group live in ONE PSUM bank at
    partition offsets 0/32, col-tiled (tile_position) so they overlap
    on the PE array; 6 PSUM banks feed the encode accumulation
  - binarize (Sign) on the Scalar engine, centering on DVE
  - hamming matmuls emitted one d-tile late so PE never waits on the
    binarize; epilogue on Scalar, output DMA per b-block immediately
"""

import sys

sys.path.insert(0, "/opt/trn_rl_repo")

import numpy as np

import concourse.bacc as bacc
import concourse.bass as bass
import concourse.mybir as mybir
import concourse.tile as tile
from concourse.bass_utils import run_bass_kernel_spmd

B = 4096
F = 784
D = 10000
C = 10
NCORES = 8
DREAL = D // NCORES          # 1250 real dims per core
DP = 1280                    # padded to 10 d-tiles of 128
ND = DP // 128               # 10
NB = B // 512                # 8 b-blocks of 512
FT = [(i * 128, min(128, F - i * 128)) for i in range((F + 127) // 128)]
NF = len(FT)                 # 7 (6x128 + 16)
NWARM = 28                   # PE warmup matmuls
NFILL = 5                    # dummy matmuls per fi-step of the first group

F32 = mybir.dt.float32
F32R = mybir.dt.float32r
BF16 = mybir.dt.bfloat16
OP = mybir.AluOpType
AF = mybir.ActivationFunctionType

_NC_CACHE = {}


def _build_nc():
    if "nc" in _NC_CACHE:
        return _NC_CACHE["nc"]
    nc = bacc.Bacc("TRN2", debug=False, target_bir_lowering=False)
    xT = nc.dram_tensor("xT", [F, B], F32R, kind="ExternalInput")
    wT = nc.dram_tensor("wT", [F, DP], F32R, kind="ExternalInput")
    cT = nc.dram_tensor("cT", [DP, C], F32, kind="ExternalInput")
    out = nc.dram_tensor("out", [C, B], F32, kind="ExternalOutput")

    with tile.TileContext(nc) as tc:
        with (
            tc.tile_pool(name="dum", bufs=2) as dumpool,
            tc.tile_pool(name="xp", bufs=NB // 2 * NF) as xpool,
            tc.tile_pool(name="wp", bufs=(ND + 1) // 2 * NF) as wpool,
            tc.tile_pool(name="cp", bufs=1) as cpool,
            tc.tile_pool(name="cmp", bufs=1) as cmpool,
            tc.tile_pool(name="ep", bufs=6) as epool,
            tc.tile_pool(name="op", bufs=4) as opool,
            tc.tile_pool(name="pse", bufs=4, space="PSUM") as psepool,
            tc.tile_pool(name="ps2", bufs=4, space="PSUM") as ps2pool,
        ):
            # --- PE warmup: release the HAM clock gate while inputs load.
            wdum = dumpool.tile([128, 128], BF16)
            nc.gpsimd.memset(wdum[:], 1.0)
            xdum = dumpool.tile([128, 512], BF16)
            nc.gpsimd.memset(xdum[:], 1.0)
            psdum = psepool.tile([128, 512], F32, name="psdum", tag="pse")
            for i in range(NWARM):
                nc.tensor.matmul(psdum[:], wdum[:], xdum[:],
                                 start=(i == 0), stop=(i == NWARM - 1))

            # --- centroid prep: one DMA for all 10 d-tiles, then
            # cmod = 1 - 2c (bf16). Pad rows are 0.5 -> cmod = 0.
            ct = cpool.tile([128, ND * C], F32)
            nc.gpsimd.dma_start(
                ct[:].rearrange("p (a c) -> p a c", c=C),
                cT.ap().rearrange("(a p) c -> p a c", p=128))
            cmod = cmpool.tile([128, ND * C], BF16)
            nc.scalar.activation(cmod[:], ct[:], AF.Copy, bias=1.0,
                                 scale=-2.0)
            cmods = [cmod[:, di * C:(di + 1) * C] for di in range(ND)]

            # --- input loads; tiles single-assignment (loaded once, no
            # slot reuse) so input DMAs never carry data waits. x tiles
            # span two b-blocks, w tiles two d-tiles.
            xts = {}
            wts = {}

            def load_x(bp, fi):   # bp = b-block pair index (0..3)
                f0, fl = FT[fi]
                xt = xpool.tile([fl, 1024], F32R, name=f"xt_{bp}_{fi}",
                                tag="xt")
                nc.sync.dma_start(
                    xt[:], xT[f0:f0 + fl, bp * 1024:(bp + 1) * 1024])
                # center (x - 0.5) in place on DVE
                nc.vector.tensor_scalar_add(xt[:], xt[:], -0.5)
                xts[bp, fi] = xt

            def load_w(dp, fi):   # dp = d-tile pair index (0..4)
                f0, fl = FT[fi]
                wid = min(256, DP - dp * 256)
                wt = wpool.tile([fl, wid], F32R, name=f"wt_{dp}_{fi}",
                                tag="wt")
                nc.gpsimd.dma_start(
                    wt[:], wT[f0:f0 + fl, dp * 256:dp * 256 + wid])
                wts[dp, fi] = wt

            for i in range(5):
                for fi in range(NF):
                    if i < 4:
                        load_x(i, fi)
                    load_w(i, fi)

            def xop(bb, fi):
                return xts[bb // 2, fi][:, (bb % 2) * 512:(bb % 2 + 1) * 512]

            def wop(di, fi):
                return wts[di // 2, fi][:, (di % 2) * 128:(di % 2 + 1) * 128]

            # --- main compute: two b-groups of 4 blocks.
            for bg in range(2):
                bbs = list(range(bg * 4, bg * 4 + 4))
                psum2 = {}
                for bb in bbs:
                    psum2[bb] = ps2pool.tile([C, 512], F32,
                                             name=f"ps2_{bb}", tag="ps2")
                pending = []
                for di in range(ND):
                    pses = {}
                    for bb in bbs:
                        pses[bb] = psepool.tile([128, 512], F32,
                                                name=f"pse_{di % 2}_{bb}",
                                                tag="pse")
                    for fi in range(NF):
                        for bb in bbs:
                            nc.tensor.matmul(pses[bb][:], wop(di, fi),
                                             xop(bb, fi),
                                             start=(fi == 0),
                                             stop=(fi == NF - 1))
                    ets = {}
                    for bb in bbs:
                        # e' = (hv > 0) - 0.5 in {-1/2, +1/2}
                        et = epool.tile([128, 512], BF16,
                                        name=f"et_{di % 2}_{bb}", tag="et")
                        nc.vector.tensor_scalar(et[:], pses[bb][:], 0.0,
                                                0.5, op0=OP.is_gt,
                                                op1=OP.subtract)
                        ets[bb] = et
                    for pdi, pbb, pet in pending:
                        nc.tensor.matmul(psum2[pbb][:], cmods[pdi],
                                         pet[:], start=(pdi == 0),
                                         stop=(pdi == ND - 1))
                    pending = [(di, bb, ets[bb]) for bb in bbs]
                for pdi, pbb, pet in pending:
                    nc.tensor.matmul(psum2[pbb][:], cmods[pdi], pet[:],
                                     start=(pdi == 0), stop=(pdi == ND - 1))
                    # out = -(psum2 + DREAL/2), on the Scalar engine
                    ot = opool.tile([C, 512], F32, name=f"ot_{pbb % 4}",
                                    tag="ot")
                    nc.scalar.activation(ot[:], psum2[pbb][:], AF.Copy,
                                         bias=-float(DREAL) / 2.0,
                                         scale=-1.0)
                    nc.gpsimd.dma_start(
                        out[:, pbb * 512:(pbb + 1) * 512], ot[:])
    nc.compile()
    _NC_CACHE["nc"] = nc
    return nc


def _prep_in_maps(samples, bhv_matrix, centroids):
    samples = np.ascontiguousarray(samples, dtype=np.float32)
    bhv_matrix = np.ascontiguousarray(bhv_matrix, dtype=np.float32)
    centroids = np.ascontiguousarray(centroids, dtype=np.float32)
    xT = np.ascontiguousarray(samples.T)  # [F, B]
    in_maps = []
    for k in range(NCORES):
        lo_, hi_ = k * DREAL, (k + 1) * DREAL
        wTk = np.zeros((F, DP), dtype=np.float32)
        wTk[:, :DREAL] = bhv_matrix[lo_:hi_, :].T
        cTk = np.full((DP, C), 0.5, dtype=np.float32)
        cTk[:DREAL, :] = centroids[:, lo_:hi_].T
        in_maps.append({"xT": xT, "wT": wTk, "cT": cTk})
    return in_maps


def _run(samples, bhv_matrix, centroids, **spmd_kwargs):
    nc = _build_nc()
    in_maps = _prep_in_maps(samples, bhv_matrix, centroids)
    res = run_bass_kernel_spmd(nc, in_maps, core_ids=list(range(NCORES)),
                               **spmd_kwargs)
    acc = np.zeros((C, B), dtype=np.float32)
    for r in res.results:
        acc += r["out"]
    return np.ascontiguousarray(acc.T), res


def kernel(samples, bhv_matrix, centroids):
    out, _ = _run(samples, bhv_matrix, centroids)
    return out
